# revision 63
# baseline (speedup 1.0000x reference)
"""Trainium2 Bass kernel for nn_DeltaModel (scatter_memory).

Algorithm: every per-token quantity (embedding -> MLP -> LayerNorm -> k/v/q
projections) is a pure function of the vocab id (V=64), so the encode collapses
to 64-row tables computed once on device.  The delta-rule scan
    M_{t+1} = M_t + (v_t - M_t k_t) k_t^T ,  out = M_T q
collapses (since only M_T @ q is needed) to a backward vector recursion
    u <- q;  for t = T-1..0:  a_t = k_t . u ;  u <- u - a_t k_t
    M_T q = sum_t a_t v_t
On device: k_t / v_t rows are indirect-DMA row-gathers from the tables by seq
ids; the recursion runs on the Vector engine (2 fused ops per step, batch on
partitions); the answer sum runs as per-chunk PE matmuls accumulated in PSUM.
Sign trick: the update is computed as u' = (k*a) - u (hardware op order), which
flips the sign of u every step; the stored a_t alternate sign accordingly and
are corrected by a +/-1 parity scale folded into the answer matmuls.

Schedule notes (v2): the DVE row chain (2 dependent ops x 2047 steps at the
semaphore-limited ~444ns/row) is the critical path and sits at this machine's
floor; everything else must hide under it.  Changes vs v1: constant loads are
issued via the Scalar engine's HWDGE path so the Pool queue (whose serial
~1.3us/call SWDGE descriptor generation gates the k-row gather pipeline)
reaches the gathers immediately; k-row gathers are software-pipelined one
supertile ahead; the k-table build is ordered first in setup, with the MLP
hidden layer computed transposed to skip a transpose roundtrip; the first
NBRIDGE time-slots are built on-chip via one-hot PE matmuls (dodging the HBM
gather latency at the start of the sweep); q-row init uses a one-hot matmul
instead of an HBM gather roundtrip; Sqrt/Square activation tables are
prefetched with dummy ops so their 1.3us loads overlap the input DMAs;
per-chunk answer partials are staged to SBUF by the Scalar engine and folded
into a running accumulator on GPSIMD during the sweep, so the Vector engine
never leaves the row chain and almost no answer work remains after the last
row; Wout @ Wrp and both biases are folded on the host into a single
epilogue matmul.

Sharding: pure data parallel, batch 256 -> 8 cores x 32.
"""

import numpy as np

B, L, V, H = 256, 2048, 64, 64  # problem shape (hardcoded per spec)
NCORES = 8
BL = B // NCORES  # 32
T_FULL = L - 1  # 2047
SUPER = 128  # sweep gather tile (time steps)
CHUNK = 128  # answer-matmul chunk (time steps)

_CACHE = {}
LAST_RESULTS = None


def _build_nc(T):
    import concourse.bass as bass
    import concourse.mybir as mybir
    import concourse.tile as tile
    from concourse import bacc

    f32 = mybir.dt.float32
    i32 = mybir.dt.int32
    Alu = mybir.AluOpType
    Act = mybir.ActivationFunctionType

    nc = bacc.Bacc("TRN2", target_bir_lowering=False, debug=False,
                   num_devices=NCORES)

    # ---- I/O -----------------------------------------------------------
    TP = (T + SUPER - 1) // SUPER * SUPER  # padded step count (2048)
    NST = TP // SUPER
    NCH = TP // CHUNK
    i16 = mybir.dt.int16
    kidx_d = nc.dram_tensor("kidx", [128, NST * SUPER * 8], i16,
                            kind="ExternalInput")
    vidx_d = nc.dram_tensor("vidx", [128, NCH * BL * CHUNK // 16], i16,
                            kind="ExternalInput")
    NBRIDGE = 16  # first time-slots computed on-chip (no gather latency)
    # merged constant blocks (fewer serial HWDGE issues at startup):
    # m1 = embT | w1b1 ; m3 = eb2(padded) | wkb ; m4 = ohk0 | ohq
    m1_d = nc.dram_tensor("m1", [H + 1, V + 2 * H], f32,
                          kind="ExternalInput")
    m3_d = nc.dram_tensor("m3", [H + 1, 2 * H], f32, kind="ExternalInput")
    m4_d = nc.dram_tensor("m4", [V, (NBRIDGE + 1) * BL], f32,
                          kind="ExternalInput")
    w2T_d = nc.dram_tensor("w2T", [2 * H, H], f32, kind="ExternalInput")
    wvb_d = nc.dram_tensor("wvb", [H + 1, H], f32, kind="ExternalInput")
    wqb_d = nc.dram_tensor("wqb", [H + 1, H], f32, kind="ExternalInput")
    wcob_d = nc.dram_tensor("wcob", [H + 1, V], f32, kind="ExternalInput")
    iden_d = nc.dram_tensor("iden", [128, 128], f32, kind="ExternalInput")
    pm_d = nc.dram_tensor("pm", [128, 1], f32, kind="ExternalInput")
    out_d = nc.dram_tensor("out", [BL, V], f32, kind="ExternalOutput")

    kn_d = nc.dram_tensor("kn_scratch", [V, H], f32, kind="Internal")
    vt_d = nc.dram_tensor("vt_scratch", [V, H], f32, kind="Internal")

    with tile.TileContext(nc) as tc:
        with (
            tc.tile_pool(name="const", bufs=1) as cp,
            tc.tile_pool(name="setup", bufs=1) as sp,
            tc.tile_pool(name="setup_ps", bufs=4, space="PSUM") as spp,
            tc.tile_pool(name="sweep", bufs=1) as swp,
            tc.tile_pool(name="kst", bufs=2) as kp,
            tc.tile_pool(name="vst", bufs=3) as vp,
            tc.tile_pool(name="ans_ps", bufs=2, space="PSUM") as ap_pool,
            tc.tile_pool(name="at_ps", bufs=2, space="PSUM") as atp,
        ):
            # ---- load constants ---------------------------------------
            def load(pool, dram, shape, tag, dtype=f32):
                t = pool.tile(shape, dtype, tag=tag)
                nc.gpsimd.dma_start(out=t[:], in_=dram.ap())
                return t

            def load_on(eng, pool, dram, shape, tag, dtype=f32):
                t = pool.tile(shape, dtype, tag=tag)
                eng.dma_start(out=t[:], in_=dram.ap())
                return t

            # k-table dependency chain first: the row sweep can't start
            # until kn_scratch is in HBM and the first gather lands.
            # Loads spread across SP/Act HWDGE queues + Pool SWDGE so the
            # chain's inputs land in parallel.
            m1 = load_on(nc.scalar, cp, m1_d, [H + 1, V + 2 * H], "c_m1")
            w2T = load_on(nc.scalar, cp, w2T_d, [2 * H, H], "c_w2T")
            m3 = load_on(nc.scalar, cp, m3_d, [H + 1, 2 * H], "c_m3")
            iden = load_on(nc.scalar, cp, iden_d, [128, 128], "c_iden")
            # prefetch the Sqrt activation table while loads fly
            sqw = sp.tile([1, 2], f32)
            nc.vector.memset(sqw[:], 1.0)
            nc.scalar.activation(sqw[:, 1:], sqw[:, :1], Act.Sqrt)
            nc.scalar.activation(sqw[:, 1:], sqw[:, :1], Act.Square)
            kix0 = cp.tile([128, SUPER * 8], i16, tag="c_kix0")
            nc.gpsimd.dma_start(out=kix0[:],
                                in_=kidx_d.ap()[:, :SUPER * 8])
            m4 = load_on(nc.scalar, cp, m4_d, [V, (NBRIDGE + 1) * BL],
                         "c_m4")

            # ---- setup: shared hs-table (xt) ---------------------------
            # hidden layer computed transposed ([2H, V]) so no transpose
            # roundtrip is needed before the second matmul
            ps1 = spp.tile([2 * H, V], f32, tag="sps")
            nc.tensor.matmul(ps1[:], lhsT=m1[:, V:], rhs=m1[:, :V], start=True,
                             stop=True)
            r1 = sp.tile([2 * H, V], f32)
            nc.vector.tensor_scalar_max(r1[:], ps1[:], 0.0)

            ps3 = spp.tile([V, H], f32, tag="sps")
            nc.tensor.matmul(ps3[:], lhsT=r1[:], rhs=w2T[:], start=True,
                             stop=True)
            htab = sp.tile([V, H], f32)
            nc.vector.tensor_add(htab[:], ps3[:], m3[:V, :H])

            mu = sp.tile([V, 1], f32)
            nc.vector.tensor_reduce(mu[:], htab[:], axis=mybir.AxisListType.X,
                                    op=Alu.add)
            # xc = H*htab - sum(h) = H*(htab - mean); the H factors cancel
            # exactly via the Sqrt scale/bias below
            xc = sp.tile([V, H], f32)
            nc.vector.tensor_scalar(out=xc[:], in0=htab[:], scalar1=float(H),
                                    scalar2=mu[:], op0=Alu.mult,
                                    op1=Alu.subtract)
            sq = sp.tile([V, H], f32)
            var = sp.tile([V, 1], f32)
            nc.scalar.activation(sq[:], xc[:], Act.Square, accum_out=var[:])
            eps = sp.tile([V, 1], f32)
            nc.vector.memset(eps[:], 1e-5 * H * H)
            sig = sp.tile([V, 1], f32)
            nc.scalar.activation(sig[:], var[:], Act.Sqrt, bias=eps[:],
                                 scale=1.0 / H)
            rstd = sp.tile([V, 1], f32)
            nc.vector.reciprocal(rstd[:], sig[:])
            xcn = sp.tile([V, H], f32)
            nc.vector.tensor_scalar_mul(xcn[:], xc[:], rstd[:])

            ps4 = spp.tile([H, V], f32, tag="sps")
            nc.tensor.transpose(ps4[:], xcn[:], iden[:V, :V])
            xt = sp.tile([H + 1, V], f32)
            nc.vector.memset(xt[H:H + 1, :], 1.0)
            nc.scalar.copy(xt[:H, :], ps4[:])

            # normalized k-table -> HBM (gathers depend on this)
            kps = spp.tile([V, H], f32, tag="sps")
            nc.tensor.matmul(kps[:], lhsT=xt[:], rhs=m3[:, H:], start=True,
                             stop=True)
            ksq = sp.tile([V, H], f32)
            kn2 = sp.tile([V, 1], f32)
            nc.scalar.activation(ksq[:], kps[:], Act.Square, accum_out=kn2[:])
            knm = sp.tile([V, 1], f32)
            nc.scalar.activation(knm[:], kn2[:], Act.Sqrt)
            kiv = sp.tile([V, 1], f32)
            nc.vector.reciprocal(kiv[:], knm[:])
            kn_sb = sp.tile([V, H], f32)
            nc.vector.tensor_scalar_mul(kn_sb[:], kps[:], kiv[:])
            nc.scalar.dma_start(out=kn_d.ap(), in_=kn_sb[:])

            # first supertile's k-gathers fly while the rest of setup runs
            NPC = SUPER * 128 // 1024  # gathers per supertile
            SL = SUPER // NPC          # time slots per gather piece

            def issue_kgather(st, first_piece=0):
                if st == 0:
                    kix = kix0
                else:
                    kix = kp.tile([128, SUPER * 8], i16, tag="kix")
                    nc.gpsimd.dma_start(
                        out=kix[:], in_=kidx_d.ap()[:, st * SUPER * 8:
                                                    (st + 1) * SUPER * 8])
                kst = kp.tile([128, SUPER, H], f32, tag="kst")
                for piece in range(first_piece, NPC):
                    nc.gpsimd.dma_gather(
                        out_ap=kst[:, piece * SL:(piece + 1) * SL, :],
                        in_ap=kn_d.ap(),
                        idxs_ap=kix[:, piece * 64:(piece + 1) * 64],
                        num_idxs=1024, num_idxs_reg=1024, elem_size=H)
                return kst

            # slots >= NBRIDGE of supertile 0 come from HBM gathers
            kst_cur = issue_kgather(0, first_piece=NBRIDGE // SL)

            # u init first (its Act copy must precede the bridge copies in
            # the in-order Act queue): one-hot select of per-batch q rows
            wqb = load(cp, wqb_d, [H + 1, H], "c_wqb")
            qps = spp.tile([V, H], f32, tag="sps")
            nc.tensor.matmul(qps[:], lhsT=xt[:], rhs=wqb[:], start=True,
                             stop=True)
            qt_sb = sp.tile([V, H], f32)
            nc.scalar.copy(qt_sb[:], qps[:])
            ups = spp.tile([BL, H], f32, tag="sps")
            nc.tensor.matmul(ups[:], lhsT=m4[:, NBRIDGE * BL:], rhs=qt_sb[:], start=True,
                             stop=True)



            # slots < NBRIDGE built on-chip from the k-table (PE one-hot
            # select + Act copy), dodging the first gathers' latency
            for j in range(NBRIDGE):
                kbp = spp.tile([BL, H], f32, tag="sps")
                nc.tensor.matmul(kbp[:], lhsT=m4[:, j * BL:(j + 1) * BL],
                                 rhs=kn_sb[:], start=True, stop=True)
                nc.scalar.copy(kst_cur[:BL, j, :], kbp[:])

            # remaining setup (v table, constants) overlaps the gathers
            wvb = load(cp, wvb_d, [H + 1, H], "c_wvb")
            wcob = load(cp, wcob_d, [H + 1, V], "c_wcob")
            pm = load(cp, pm_d, [128, 1], "c_pm")
            vidx_sb = load(cp, vidx_d, [128, NCH * BL * CHUNK // 16],
                           "c_vidx", i16)

            vps = spp.tile([V, H], f32, tag="sps")
            nc.tensor.matmul(vps[:], lhsT=xt[:], rhs=wvb[:], start=True,
                             stop=True)
            vt_sb = sp.tile([V, H], f32)
            nc.scalar.copy(vt_sb[:], vps[:])
            nc.scalar.dma_start(out=vt_d.ap(), in_=vt_sb[:])

            # ---- main sweep -------------------------------------------
            u = swp.tile([BL, H], f32)
            nc.vector.tensor_copy(u[:], ups[:])
            tmp = swp.tile([BL, H], f32)
            alpha = swp.tile([BL, (T + 127) // 128 * 128], f32)
            if TP > T:  # zero only the padded tail columns
                nc.vector.memset(alpha[:, T:], 0.0)
            ans_acc = swp.tile([H, BL], f32)
            nc.vector.memset(ans_acc[:], 0.0)

            def issue_vgather(ci):
                vst = vp.tile([CHUNK, BL, H], f32, tag="vst")
                vbase = ci * BL * CHUNK // 16
                for piece in range(BL * CHUNK // 1024):
                    nc.gpsimd.dma_gather(
                        out_ap=vst[:, piece * 8:(piece + 1) * 8, :],
                        in_ap=vt_d.ap(),
                        idxs_ap=vidx_sb[:, vbase + piece * 64:
                                        vbase + (piece + 1) * 64],
                        num_idxs=1024, num_idxs_reg=1024, elem_size=H)
                return vst

            def emit_answer(blk, tau0, vst):
                # alpha[:, tau0:tau0+CHUNK] x v rows -> cpv_all block `blk`
                at_ps = atp.tile([CHUNK, BL], f32, tag="atp")
                nc.tensor.transpose(at_ps[:], alpha[:, tau0:tau0 + CHUNK],
                                    iden[:BL, :BL])
                atb = vp.tile([CHUNK, BL], f32, tag="atb")
                nc.scalar.mul(atb[:], at_ps[:], pm[:])
                cps = ap_pool.tile([H, BL], f32, tag="cps")
                for b in range(BL):
                    nc.tensor.matmul(cps[:, b:b + 1],
                                     lhsT=vst[:, b, :],
                                     rhs=atb[:, b:b + 1],
                                     start=True, stop=True)
                # stage partials via Act, then fold on Pool: the Vector
                # engine never leaves the row chain, and the prefetched
                # gathers leave Pool ~2 supertiles of slack
                cpv = vp.tile([H, BL], f32, tag="cpv")
                nc.scalar.copy(cpv[:], cps[:])
                nc.gpsimd.tensor_add(ans_acc[:], ans_acc[:], cpv[:])

            for st in range(NST):
                t0 = st * SUPER
                sc = min(SUPER, T - t0)
                kst = kst_cur
                if st + 1 < NST:
                    kst_cur = issue_kgather(st + 1)
                for j in range(sc):
                    tau = t0 + j
                    nc.vector.scalar_tensor_tensor(
                        out=tmp[:], in0=u[:], scalar=1.0, in1=kst[:BL, j, :],
                        op0=Alu.mult, op1=Alu.mult,
                        accum_out=alpha[:, tau:tau + 1])
                    nc.vector.scalar_tensor_tensor(
                        out=u[:], in0=kst[:BL, j, :],
                        scalar=alpha[:, tau:tau + 1], in1=u[:],
                        op0=Alu.mult, op1=Alu.subtract)
                vst = issue_vgather(st)
                emit_answer(st, t0, vst)

            # ---- epilogue ---------------------------------------------
            ansx = sp.tile([H + 1, BL], f32)
            nc.vector.memset(ansx[H:H + 1, :], 1.0)
            nc.scalar.copy(ansx[:H, :], ans_acc[:])
            # host folds Wout @ Wrp (and both biases) into one matrix
            # operand swap makes the output land [BL, V] directly -- no
            # final transpose roundtrip
            ops_ = spp.tile([BL, V], f32, tag="sps")
            nc.tensor.matmul(ops_[:], lhsT=ansx[:], rhs=wcob[:], start=True,
                             stop=True)
            o_fin = sp.tile([BL, V], f32)
            nc.scalar.copy(o_fin[:], ops_[:])
            nc.scalar.dma_start(out=out_d.ap(), in_=o_fin[:])

    nc.compile()
    return nc


def _marshal(inputs, T):
    f = np.float32
    seq = np.asarray(inputs["seq"])
    embed = np.asarray(inputs["embed"], f)
    W1 = np.asarray(inputs["W1"], f)
    b1 = np.asarray(inputs["b1"], f)
    W2 = np.asarray(inputs["W2"], f)
    b2 = np.asarray(inputs["b2"], f)
    gamma = np.asarray(inputs["gamma"], f)
    beta = np.asarray(inputs["beta"], f)
    Wk = np.asarray(inputs["Wk"], f)
    Wv = np.asarray(inputs["Wv"], f)
    Wq = np.asarray(inputs["Wq"], f)
    Wrp = np.asarray(inputs["Wrp"], f)
    brp = np.asarray(inputs["brp"], f)
    Wout = np.asarray(inputs["Wout"], f)
    bout = np.asarray(inputs["bout"], f)

    embT_a = np.vstack([embed.T, np.ones((1, V), f)]).astype(f)
    w1b1_a = np.vstack([W1.T, b1[None]]).astype(f)
    eb2_a = np.vstack([(embed + b2[None]), np.zeros((1, H), f)]).astype(f)
    wkb_a = np.vstack([(Wk * gamma[None]).T, (Wk @ beta)[None]]).astype(f)
    shared = {
        "m1": np.hstack([embT_a, w1b1_a]).astype(f),
        "m3": np.hstack([eb2_a, wkb_a]).astype(f),
        "w2T": np.ascontiguousarray(W2.T, f),
        "wvb": np.vstack([(Wv * gamma[None]).T, (Wv @ beta)[None]]).astype(f),
        "wqb": np.vstack([(Wq * gamma[None]).T, (Wq @ beta)[None]]).astype(f),
        "wcob": np.vstack([(Wout @ Wrp).T,
                           (Wout @ brp + bout)[None]]).astype(f),
        "iden": np.eye(128, dtype=f),
        "pm": np.where(np.arange(128) % 2 == 0, 1.0, -1.0).astype(f)[:, None],
    }
    TP = (T + SUPER - 1) // SUPER * SUPER
    NST = TP // SUPER
    NCH = TP // CHUNK

    def wrap(flat):
        n = flat.size
        w16 = np.ascontiguousarray(flat.reshape(n // 16, 16).T).astype(np.int16)
        return np.tile(w16, (8, 1))

    in_maps = []
    for c in range(NCORES):
        sl = slice(c * BL, (c + 1) * BL)
        sseq = seq[sl]
        # reversed-time ids: ids[b, tau] = seq[b, (T-1) - tau]
        ids = np.ascontiguousarray(sseq[:, T - 1::-1]).astype(np.int64)
        idsp = np.zeros((BL, TP), np.int64)
        idsp[:, :T] = ids
        # k-stream: i = slot*128 + p ; p<BL -> ids[p, t0+slot], else dummy 0
        kblocks = []
        for st in range(NST):
            blk = np.zeros((SUPER, 128), np.int64)
            blk[:, :BL] = idsp[:, st * SUPER:(st + 1) * SUPER].T
            kblocks.append(wrap(blk.reshape(-1)))
        # v-stream: i = b*128 + tau ; chunk frames of CHUNK
        vblocks = []
        for ci in range(NCH):
            blk = idsp[:, ci * CHUNK:(ci + 1) * CHUNK]  # [BL, CHUNK]
            vblocks.append(wrap(blk.reshape(-1)))
        ohq = np.zeros((V, BL), np.float32)
        ohq[sseq[:, L - 1], np.arange(BL)] = 1.0
        NBRIDGE = 16
        ohk0 = np.zeros((V, NBRIDGE * BL), np.float32)
        for j in range(NBRIDGE):
            ohk0[ids[:, j], j * BL + np.arange(BL)] = 1.0
        m = dict(shared)
        m["m4"] = np.hstack([ohk0, ohq]).astype(np.float32)
        m["kidx"] = np.concatenate(kblocks, axis=1)
        m["vidx"] = np.concatenate(vblocks, axis=1)
        in_maps.append(m)
    return in_maps


def kernel(**inputs):
    global LAST_RESULTS
    import os
    from concourse.bass_utils import run_bass_kernel_spmd

    T = T_FULL
    if "nc" not in _CACHE:
        _CACHE["nc"] = _build_nc(T)
    nc = _CACHE["nc"]
    in_maps = _marshal(inputs, T)
    trace = bool(int(os.environ.get("KERNEL_TRACE", "0")))
    res = run_bass_kernel_spmd(nc, in_maps, core_ids=list(range(NCORES)),
                               trace=trace)
    LAST_RESULTS = res
    out = np.concatenate([res.results[c]["out"] for c in range(NCORES)],
                         axis=0)
    return out.astype(np.float32)
